# revision 2
# baseline (speedup 1.0000x reference)
"""AdaptiveBlockSparseAttnTrain Trainium2 kernel (8 NeuronCores, head-parallel).

Pipeline per core (= one attention head), fused single pass over query blocks:
  - Gilbert rearrange/unrearrange + padding + transposes done host-side.
  - ST_ij = K_j @ Q_i^T on TensorE in fp32r (tf32) at full rate (N>=256).
  - E = exp(ST * scale) on ScalarE -> fp16 (zero mask flips vs f32, measured).
  - den via 31 accumulating column-sum matmuls (ones lhsT, no LDW churn).
  - pnorm = E * (1/den) broadcast (DVE), pooled A = reduce (DVE),
    pooling row via tiny matmuls, energy mask computed rank-based:
       keep[j] = (cum_incl[j] < 0.95*total & rank[j] < 21) | rank[j] < 1
    which equals the reference's argsort/cumsum/clip construction for
    tie-free inputs.
  - mask multiply in place, then PV with an appended ones column so the
    masked softmax denominator falls out of the same matmul; final row
    rescale by its reciprocal.
"""

import os
import sys

sys.path.insert(0, "/opt/trn_rl_repo")

import numpy as np

import concourse.bass as bass
import concourse.bacc as bacc
import concourse.tile as tile
from concourse import mybir
from concourse.bass_utils import run_bass_kernel_spmd

TEXT = 224
VID = 3696
SEQ = 3920
BLOCK = 128
NB = 31
SP = 3968
D = 128
NCORES = 8
VW = 132          # V row width: 128 values + ones col + 3 pad
SCALE = 1.0 / np.sqrt(128.0)

F32 = mybir.dt.float32
F32R = mybir.dt.float32r
F16 = mybir.dt.float16

# i-groups of 4 query blocks so the fp32r ST matmul free dim is >= 256
GROUPS = [(0, 4), (4, 4), (8, 4), (12, 4), (16, 4), (20, 4), (24, 4), (28, 3)]


def _bcast_ap(t_ap, mid_count, inner_count, inner_step, mid_step):
    """Build a 3-D broadcast AP [[part], [mid], [inner]] from a 2-D tile AP."""
    return bass.AP(
        tensor=t_ap.tensor,
        offset=t_ap.offset,
        ap=[list(t_ap.ap[0]), [mid_step, mid_count], [inner_step, inner_count]],
    )


def build_graph():
    nc = bacc.Bacc("TRN2", target_bir_lowering=False, debug=False,
                   num_devices=NCORES)
    qT_d = nc.dram_tensor("qT", [128, SP], F32, kind="ExternalInput").ap()
    kT_d = nc.dram_tensor("kT", [128, SP], F32, kind="ExternalInput").ap()
    vv_d = nc.dram_tensor("vv", [128, NB * VW], F16, kind="ExternalInput").ap()
    i31_d = nc.dram_tensor("i31", [31, 31], F32, kind="ExternalInput").ap()
    out_d = nc.dram_tensor("out", [SP, 128], F32, kind="ExternalOutput").ap()

    with tile.TileContext(nc) as tc:
        with (
            tc.tile_pool(name="singles", bufs=1) as singles,
            tc.tile_pool(name="eg", bufs=2) as egp,
            tc.tile_pool(name="pn", bufs=3) as pnp,
            tc.tile_pool(name="small", bufs=4) as small,
            tc.tile_pool(name="outs", bufs=3) as outsp,
            tc.tile_pool(name="stps", bufs=2, space="PSUM") as stps,
            tc.tile_pool(name="pvps", bufs=2, space="PSUM") as pvps,
            tc.tile_pool(name="mini", bufs=3, space="PSUM") as minips,
        ):
            # ---- resident inputs ----
            sq = singles.tile([128, SP], F32R)
            sk = singles.tile([128, SP], F32R)
            sv = singles.tile([128, NB, VW], F16)
            i31 = singles.tile([31, 31], F32)
            nc.sync.dma_start(sq[:, :], qT_d.bitcast(F32R))
            nc.sync.dma_start(sk[:, :], kT_d.bitcast(F32R))
            nc.sync.dma_start(sv[:, :, :], vv_d.rearrange("p (j w) -> p j w", j=NB))
            nc.sync.dma_start(i31[:, :], i31_d)

            # ---- constants ----
            ones_col16 = singles.tile([128, 1], F16)   # den matmul lhsT
            nc.vector.memset(ones_col16[:, :], 1.0)
            ones_col32 = singles.tile([128, 1], F32)   # pooling matmul rhs
            nc.vector.memset(ones_col32[:, :], 1.0)
            ones_row128 = singles.tile([1, 128], F32)  # broadcast matmul lhsT
            nc.vector.memset(ones_row128[:, :], 1.0)
            ones_row31 = singles.tile([1, 31], F32)    # Pb broadcast lhsT
            nc.vector.memset(ones_row31[:, :], 1.0)
            ones_1 = singles.tile([1, 1], F32)
            nc.vector.memset(ones_1[:, :], 1.0)

            for (i0, G) in GROUPS:
                GW = G * 128
                eg = egp.tile([128, NB, GW], F16, tag="eg")
                # ---- scores + exp for the whole group ----
                for j in range(NB):
                    st = stps.tile([128, GW], F32, tag="st")
                    nc.tensor.matmul(
                        st[:, :],
                        sk[:, j * 128:(j + 1) * 128],
                        sq[:, i0 * 128:i0 * 128 + GW],
                        start=True, stop=True,
                    )
                    nc.scalar.activation(
                        eg[:, j, :], st[:, :],
                        mybir.ActivationFunctionType.Exp,
                        bias=0.0, scale=float(SCALE),
                    )
                # zero invalid keys (rows 96..127 of key block 30)
                nc.vector.memset(eg[96:128, 30, :], 0.0)

                for il in range(G):
                    i = i0 + il
                    nv = 96 if i == 30 else 128
                    qs0 = il * 128

                    # ---- den: accumulate column sums over all key blocks ----
                    den_ps = minips.tile([1, 128], F32, tag="mini")
                    for j in range(NB):
                        nc.tensor.matmul(
                            den_ps[:, :nv],
                            ones_col16[:, :],
                            eg[:, j, qs0:qs0 + nv],
                            start=(j == 0), stop=(j == NB - 1),
                        )
                    rdw_row = small.tile([1, 128], F32, tag="rdwrow")
                    nc.vector.reciprocal(rdw_row[:, :nv], den_ps[:, :nv])

                    # ---- broadcast 1/den across partitions, cast fp16 ----
                    rb_ps = minips.tile([128, 128], F32, tag="mini")
                    nc.tensor.matmul(rb_ps[:, :nv], ones_row128[:, :],
                                     rdw_row[:, :nv], start=True, stop=True)
                    rdw_bc = small.tile([128, 128], F16, tag="rdwbc")
                    nc.vector.tensor_copy(rdw_bc[:, :nv], rb_ps[:, :nv])

                    # ---- pnorm = E * rdw (per-query normalized probs) ----
                    pn = pnp.tile([128, NB, 128], F16, tag="pn")
                    if nv < 128:
                        nc.vector.memset(pn[:, :, nv:], 0.0)
                    nc.vector.tensor_tensor(
                        pn[:, :, :nv],
                        eg[:, :, qs0:qs0 + nv],
                        _bcast_ap(rdw_bc[:, :nv], NB, nv,
                                  inner_step=1, mid_step=0),
                        mybir.AluOpType.mult,
                    )

                    # ---- pooled row: A = sum_q pnorm ; P = ones^T A ----
                    A = small.tile([128, NB], F32, tag="A")
                    nc.vector.reduce_sum(A[:, :], pn[:, :, :nv],
                                         axis=mybir.AxisListType.X)
                    pcol_ps = minips.tile([31, 1], F32, tag="mini")
                    nc.tensor.matmul(pcol_ps[:, :], A[:, :], ones_col32[:, :],
                                     start=True, stop=True)
                    pcol = small.tile([31, 1], F32, tag="pcol")
                    nc.vector.tensor_copy(pcol[:, :], pcol_ps[:, :])

                    # ---- Prow / Pb ----
                    prow_ps = minips.tile([1, 31], F32, tag="mini")
                    nc.tensor.matmul(prow_ps[:, :], pcol[:, :], i31[:, :],
                                     start=True, stop=True)
                    prow = small.tile([1, 31], F32, tag="prow")
                    nc.vector.tensor_copy(prow[:, :], prow_ps[:, :])
                    pb_ps = minips.tile([31, 31], F32, tag="mini")
                    nc.tensor.matmul(pb_ps[:, :], ones_row31[:, :], prow[:, :],
                                     start=True, stop=True)

                    # ---- energy mask, rank formulation ----
                    pb = small.tile([31, 31], F32, tag="pb")
                    nc.vector.tensor_copy(pb[:, :], pb_ps[:, :])
                    Gt = small.tile([31, 31], F32, tag="Gt")
                    nc.vector.tensor_scalar(Gt[:, :], pb[:, :], pcol[:, :],
                                            None, mybir.AluOpType.is_gt)
                    rank = small.tile([31, 1], F32, tag="rank")
                    nc.vector.reduce_sum(rank[:, :], Gt[:, :],
                                         axis=mybir.AxisListType.X)
                    Geq = small.tile([31, 31], F32, tag="Geq")
                    nc.vector.tensor_scalar(Geq[:, :], pb[:, :], pcol[:, :],
                                            None, mybir.AluOpType.is_ge)
                    tmp3131 = small.tile([31, 31], F32, tag="tmp3131")
                    nc.vector.tensor_tensor(tmp3131[:, :], Geq[:, :], pb[:, :],
                                            mybir.AluOpType.mult)
                    esum = small.tile([31, 1], F32, tag="esum")
                    nc.vector.reduce_sum(esum[:, :], tmp3131[:, :],
                                         axis=mybir.AxisListType.X)
                    tot = small.tile([31, 1], F32, tag="tot")
                    nc.vector.reduce_sum(tot[:, :], pb[:, :],
                                         axis=mybir.AxisListType.X)
                    C = small.tile([31, 1], F32, tag="C")
                    nc.vector.scalar_tensor_tensor(
                        C[:, :], tot[:, :], 0.95, esum[:, :],
                        mybir.AluOpType.mult, mybir.AluOpType.is_gt,
                    )
                    r21 = small.tile([31, 1], F32, tag="r21")
                    nc.vector.tensor_single_scalar(r21[:, :], rank[:, :], 21.0,
                                                   mybir.AluOpType.is_lt)
                    ca = small.tile([31, 1], F32, tag="ca")
                    nc.vector.tensor_tensor(ca[:, :], C[:, :], r21[:, :],
                                            mybir.AluOpType.logical_and)
                    r1 = small.tile([31, 1], F32, tag="r1")
                    nc.vector.tensor_single_scalar(r1[:, :], rank[:, :], 1.0,
                                                   mybir.AluOpType.is_lt)
                    mv = small.tile([31, 1], F32, tag="mv")
                    nc.vector.tensor_tensor(mv[:, :], ca[:, :], r1[:, :],
                                            mybir.AluOpType.logical_or)

                    # ---- broadcast mask across partitions, cast fp16 ----
                    mrow_ps = minips.tile([1, 31], F32, tag="mini")
                    nc.tensor.matmul(mrow_ps[:, :], mv[:, :], i31[:, :],
                                     start=True, stop=True)
                    mrow = small.tile([1, 31], F32, tag="mrow")
                    nc.vector.tensor_copy(mrow[:, :], mrow_ps[:, :])
                    mb_ps = minips.tile([128, 31], F32, tag="mini")
                    nc.tensor.matmul(mb_ps[:, :], ones_row128[:, :],
                                     mrow[:, :], start=True, stop=True)
                    mbc = small.tile([128, 31], F16, tag="mbc")
                    nc.vector.tensor_copy(mbc[:, :], mb_ps[:, :])

                    # ---- apply mask in place ----
                    nc.vector.tensor_tensor(
                        pn[:, :, :],
                        pn[:, :, :],
                        _bcast_ap(mbc[:, :], NB, 128,
                                  inner_step=0, mid_step=1),
                        mybir.AluOpType.mult,
                    )

                    # ---- PV with appended ones column ----
                    o_ps = pvps.tile([128, VW], F32, tag="ops")
                    for j in range(NB):
                        nc.tensor.matmul(
                            o_ps[:, :],
                            pn[:, j, :],
                            sv[:, j, :],
                            start=(j == 0), stop=(j == NB - 1),
                        )
                    ro = small.tile([128, 1], F32, tag="ro")
                    nc.vector.reciprocal(ro[:, :], o_ps[:, 128:129])
                    oout = outsp.tile([128, 128], F32, tag="oout")
                    nc.vector.tensor_scalar(oout[:, :], o_ps[:, 0:128],
                                            ro[:, :], None,
                                            mybir.AluOpType.mult)
                    nc.sync.dma_start(out_d[i * 128:(i + 1) * 128, :],
                                      oout[:, :])

    nc.compile()
    return nc


_CACHED = {}


def _get_graph():
    if "nc" not in _CACHED:
        _CACHED["nc"] = build_graph()
    return _CACHED["nc"]


def _prepare_inputs(q, k, v, perm):
    q = np.asarray(q, dtype=np.float32)
    k = np.asarray(k, dtype=np.float32)
    v = np.asarray(v, dtype=np.float32)
    perm = np.asarray(perm, dtype=np.int64)

    def rearr(x):  # [1,8,SEQ,D] -> video permuted first, text appended
        return np.concatenate([x[0, :, TEXT:, :][:, perm, :], x[0, :, :TEXT, :]],
                              axis=1)

    qr, kr, vr = rearr(q), rearr(k), rearr(v)      # [8, SEQ, D]
    pad = SP - SEQ
    in_maps = []
    i31 = np.eye(31, dtype=np.float32)
    for c in range(NCORES):
        qp = np.zeros((SP, D), np.float32)
        qp[:SEQ] = qr[c]
        kp = np.zeros((SP, D), np.float32)
        kp[:SEQ] = kr[c]
        vp = np.zeros((SP, D), np.float32)
        vp[:SEQ] = vr[c]
        qT = np.ascontiguousarray(qp.T)
        kT = np.ascontiguousarray(kp.T)
        vvc = np.zeros((128, NB, VW), np.float16)
        vvc[:, :, :128] = vp.reshape(NB, 128, D).transpose(1, 0, 2)
        vvc[:, :, 128] = 1.0
        in_maps.append({
            "qT": qT,
            "kT": kT,
            "vv": np.ascontiguousarray(vvc.reshape(128, NB * VW)),
            "i31": i31,
        })
    return in_maps, perm


def run(inputs, trace=False, trace_kwargs=None):
    nc = _get_graph()
    in_maps, perm = _prepare_inputs(inputs["q"], inputs["k"], inputs["v"],
                                    inputs["perm"])
    res = run_bass_kernel_spmd(
        nc, in_maps, core_ids=list(range(NCORES)), trace=trace,
        **(trace_kwargs or {}),
    )
    outs = np.stack([res.results[c]["out"][:SEQ] for c in range(NCORES)])
    g2o = np.argsort(perm)
    txt = outs[:, VID:SEQ, :]
    vid = outs[:, :VID, :][:, g2o, :]
    full = np.concatenate([txt, vid], axis=1)[None]   # [1, 8, SEQ, D]
    return np.ascontiguousarray(full.astype(np.float32)), res


def kernel(q, k, v, perm):
    out, _ = run({"q": q, "k": k, "v": v, "perm": perm})
    return out


# revision 4
# speedup vs baseline: 1.6928x; 1.6928x over previous
"""AdaptiveBlockSparseAttnTrain Trainium2 kernel (8 NeuronCores, head-parallel).

Per core (= one head), fused single pass over query-block groups:
  - Gilbert rearrange/unrearrange, padding, transposes, final division done
    host-side (cheap numpy); device computes the attention pipeline.
  - ST_ij = K_j @ Q_group^T on TensorE in fp16 (same precision class as tf32;
    measured zero energy-mask flips vs the f32 reference).
  - E = exp(ST * scale) on ScalarE -> fp16.
  - W[j, q] = sum_r E_ij[r, q] via 31 accumulating basis matmuls per group
    (the basis constant also encodes key validity for the partial last block).
  - den[q] = ones^T W, pooling row P_i[j] = sum_q W[j,q]/den[q] via tiny
    matmuls + a PE transpose of W's i-slice.
  - Energy mask, rank-based (equals reference argsort/cumsum/clip for
    tie-free inputs):  keep = (cum_incl < 0.95*tot & rank < 21) | rank < 1.
  - Mask multiply in place on E (one DVE pass per query block).
  - PV transposed and group-batched: O^T_group[d, q] += V_j^T @ E_masked_j
    (31 matmuls of N=512 per group; V stationary).
  - Masked denominator row der_m = m^T W via a tiny matmul; host divides.
"""

import sys

sys.path.insert(0, "/opt/trn_rl_repo")

import numpy as np

import concourse.bass as bass
import concourse.bacc as bacc
import concourse.tile as tile
from concourse import mybir
from concourse.bass_utils import run_bass_kernel_spmd

TEXT = 224
VID = 3696
SEQ = 3920
BLOCK = 128
NB = 31
SP = 3968
D = 128
NCORES = 8
NVLAST = SEQ - 30 * 128        # 80 valid tokens in the last block
SCALE = 1.0 / np.sqrt(128.0)

F32 = mybir.dt.float32
F16 = mybir.dt.float16

GROUPS = [(0, 4), (4, 4), (8, 4), (12, 4), (16, 4), (20, 4), (24, 4), (28, 3)]


def _bcast_ap(t_ap, mid_count, inner_count, inner_step, mid_step):
    """3-D broadcast AP [[part], [mid], [inner]] from a 2-D tile AP."""
    return bass.AP(
        tensor=t_ap.tensor,
        offset=t_ap.offset,
        ap=[list(t_ap.ap[0]), [mid_step, mid_count], [inner_step, inner_count]],
    )


def build_graph():
    nc = bacc.Bacc("TRN2", target_bir_lowering=False, debug=False,
                   num_devices=NCORES)
    qT_d = nc.dram_tensor("qT", [128, SP], F16, kind="ExternalInput").ap()
    kT_d = nc.dram_tensor("kT", [128, SP], F16, kind="ExternalInput").ap()
    vv_d = nc.dram_tensor("vv", [128, NB * 128], F16, kind="ExternalInput").ap()
    bas_d = nc.dram_tensor("bas", [128, NB * NB], F16, kind="ExternalInput").ap()
    i31_d = nc.dram_tensor("i31", [31, 31], F32, kind="ExternalInput").ap()
    outT_d = nc.dram_tensor("outT", [128, SP], F32, kind="ExternalOutput").ap()
    den_d = nc.dram_tensor("den", [1, SP], F32, kind="ExternalOutput").ap()

    with tile.TileContext(nc) as tc:
        with (
            tc.tile_pool(name="singles", bufs=1) as singles,
            tc.tile_pool(name="eg", bufs=2) as egp,
            tc.tile_pool(name="gw", bufs=2) as gwp,
            tc.tile_pool(name="small", bufs=4) as small,
            tc.tile_pool(name="outs", bufs=2) as outsp,
            tc.tile_pool(name="stps", bufs=2, space="PSUM") as stps,
            tc.tile_pool(name="wps", bufs=1, space="PSUM") as wps,
            tc.tile_pool(name="pvps", bufs=2, space="PSUM") as pvps,
            tc.tile_pool(name="mini", bufs=3, space="PSUM") as minips,
        ):
            # ---- resident inputs ----
            sq = singles.tile([128, SP], F16)
            sk = singles.tile([128, SP], F16)
            sv = singles.tile([128, NB, 128], F16)
            sbas = singles.tile([128, NB, NB], F16)
            i31 = singles.tile([31, 31], F32)
            nc.sync.dma_start(sq[:, :], qT_d)
            nc.sync.dma_start(sk[:, :], kT_d)
            nc.sync.dma_start(sv[:, :, :], vv_d.rearrange("p (j w) -> p j w", j=NB))
            nc.sync.dma_start(sbas[:, :, :], bas_d.rearrange("p (j m) -> p j m", j=NB))
            nc.sync.dma_start(i31[:, :], i31_d)

            # ---- constants ----
            ones31_col = singles.tile([31, 1], F32)
            nc.vector.memset(ones31_col[:, :], 1.0)
            ones_row128 = singles.tile([1, 128], F32)
            nc.vector.memset(ones_row128[:, :], 1.0)
            ones_row31 = singles.tile([1, 31], F32)
            nc.vector.memset(ones_row31[:, :], 1.0)
            den_sb = singles.tile([1, SP], F32)

            for (i0, G) in GROUPS:
                GW = G * 128
                eg = egp.tile([128, NB, GW], F16, tag="eg")
                # ---- scores + exp ----
                for j in range(NB):
                    st = stps.tile([128, GW], F32, tag="st")
                    nc.tensor.matmul(
                        st[:, :],
                        sk[:, j * 128:(j + 1) * 128],
                        sq[:, i0 * 128:i0 * 128 + GW],
                        start=True, stop=True,
                    )
                    nc.scalar.activation(
                        eg[:, j, :], st[:, :],
                        mybir.ActivationFunctionType.Exp,
                        bias=0.0, scale=float(SCALE),
                    )

                # ---- W[j, q] = sum_r E_ij[r, q] (key-valid rows only) ----
                w_ps = wps.tile([31, GW], F32, tag="wps")
                for j in range(NB):
                    nc.tensor.matmul(
                        w_ps[:, :],
                        sbas[:, j, :],
                        eg[:, j, :],
                        start=(j == 0), stop=(j == NB - 1),
                    )
                w_sb = gwp.tile([31, GW], F32, tag="wsb")
                nc.vector.tensor_copy(w_sb[:, :], w_ps[:, :])

                for il in range(G):
                    i = i0 + il
                    nv = NVLAST if i == 30 else 128
                    qs0 = il * 128

                    # ---- den (column) and 1/den ----
                    dcol_ps = minips.tile([128, 1], F32, tag="mini")
                    nc.tensor.matmul(dcol_ps[:nv, :], w_sb[:, qs0:qs0 + nv],
                                     ones31_col[:, :], start=True, stop=True)
                    rdw = small.tile([128, 1], F32, tag="rdw")
                    nc.vector.reciprocal(rdw[:nv, :], dcol_ps[:nv, :])

                    # ---- W^T slice and pooling column ----
                    wt_ps = minips.tile([128, 31], F32, tag="mini")
                    nc.tensor.matmul(wt_ps[:nv, :], w_sb[:, qs0:qs0 + nv],
                                     i31[:, :], start=True, stop=True)
                    wt_sb = small.tile([128, 31], F32, tag="wtsb")
                    nc.vector.tensor_copy(wt_sb[:nv, :], wt_ps[:nv, :])
                    pcol_ps = minips.tile([31, 1], F32, tag="mini")
                    nc.tensor.matmul(pcol_ps[:, :], wt_sb[:nv, :], rdw[:nv, :],
                                     start=True, stop=True)
                    pcol = small.tile([31, 1], F32, tag="pcol")
                    nc.vector.tensor_copy(pcol[:, :], pcol_ps[:, :])

                    # ---- Prow / Pb ----
                    prow_ps = minips.tile([1, 31], F32, tag="mini")
                    nc.tensor.matmul(prow_ps[:, :], pcol[:, :], i31[:, :],
                                     start=True, stop=True)
                    prow = small.tile([1, 31], F32, tag="prow")
                    nc.vector.tensor_copy(prow[:, :], prow_ps[:, :])
                    pb_ps = minips.tile([31, 31], F32, tag="mini")
                    nc.tensor.matmul(pb_ps[:, :], ones_row31[:, :], prow[:, :],
                                     start=True, stop=True)
                    pb = small.tile([31, 31], F32, tag="pb")
                    nc.vector.tensor_copy(pb[:, :], pb_ps[:, :])

                    # ---- energy mask, rank formulation ----
                    Gt = small.tile([31, 31], F32, tag="Gt")
                    nc.vector.tensor_scalar(Gt[:, :], pb[:, :], pcol[:, :],
                                            None, mybir.AluOpType.is_gt)
                    rank = small.tile([31, 1], F32, tag="rank")
                    nc.vector.reduce_sum(rank[:, :], Gt[:, :],
                                         axis=mybir.AxisListType.X)
                    Geq = small.tile([31, 31], F32, tag="Geq")
                    nc.vector.tensor_scalar(Geq[:, :], pb[:, :], pcol[:, :],
                                            None, mybir.AluOpType.is_ge)
                    tmp3131 = small.tile([31, 31], F32, tag="tmp3131")
                    nc.vector.tensor_tensor(tmp3131[:, :], Geq[:, :], pb[:, :],
                                            mybir.AluOpType.mult)
                    esum = small.tile([31, 1], F32, tag="esum")
                    nc.vector.reduce_sum(esum[:, :], tmp3131[:, :],
                                         axis=mybir.AxisListType.X)
                    tot = small.tile([31, 1], F32, tag="tot")
                    nc.vector.reduce_sum(tot[:, :], pb[:, :],
                                         axis=mybir.AxisListType.X)
                    C = small.tile([31, 1], F32, tag="C")
                    nc.vector.scalar_tensor_tensor(
                        C[:, :], tot[:, :], 0.95, esum[:, :],
                        mybir.AluOpType.mult, mybir.AluOpType.is_gt,
                    )
                    r21 = small.tile([31, 1], F32, tag="r21")
                    nc.vector.tensor_single_scalar(r21[:, :], rank[:, :], 21.0,
                                                   mybir.AluOpType.is_lt)
                    ca = small.tile([31, 1], F32, tag="ca")
                    nc.vector.tensor_tensor(ca[:, :], C[:, :], r21[:, :],
                                            mybir.AluOpType.logical_and)
                    r1 = small.tile([31, 1], F32, tag="r1")
                    nc.vector.tensor_single_scalar(r1[:, :], rank[:, :], 1.0,
                                                   mybir.AluOpType.is_lt)
                    mv = small.tile([31, 1], F32, tag="mv")
                    nc.vector.tensor_tensor(mv[:, :], ca[:, :], r1[:, :],
                                            mybir.AluOpType.logical_or)

                    # ---- masked denominator row (host divides by it) ----
                    dm_ps = minips.tile([1, 128], F32, tag="mini")
                    nc.tensor.matmul(dm_ps[:, :nv], mv[:, :],
                                     w_sb[:, qs0:qs0 + nv],
                                     start=True, stop=True)
                    nc.vector.tensor_copy(den_sb[:, i * 128:i * 128 + nv],
                                          dm_ps[:, :nv])

                    # ---- broadcast mask across partitions, cast fp16 ----
                    mrow_ps = minips.tile([1, 31], F32, tag="mini")
                    nc.tensor.matmul(mrow_ps[:, :], mv[:, :], i31[:, :],
                                     start=True, stop=True)
                    mrow = small.tile([1, 31], F32, tag="mrow")
                    nc.vector.tensor_copy(mrow[:, :], mrow_ps[:, :])
                    mb_ps = minips.tile([128, 31], F32, tag="mini")
                    nc.tensor.matmul(mb_ps[:, :], ones_row128[:, :],
                                     mrow[:, :], start=True, stop=True)
                    mbc = small.tile([128, 31], F16, tag="mbc")
                    nc.vector.tensor_copy(mbc[:, :], mb_ps[:, :])

                    # ---- apply mask in place on this i's E slice ----
                    nc.vector.tensor_tensor(
                        eg[:, :, qs0:qs0 + 128],
                        eg[:, :, qs0:qs0 + 128],
                        _bcast_ap(mbc[:, :], NB, 128,
                                  inner_step=0, mid_step=1),
                        mybir.AluOpType.mult,
                    )

                # ---- PV transposed, group-batched (V stationary) ----
                ot_ps = pvps.tile([128, GW], F32, tag="otps")
                for j in range(NB):
                    nc.tensor.matmul(
                        ot_ps[:, :],
                        sv[:, j, :],
                        eg[:, j, :],
                        start=(j == 0), stop=(j == NB - 1),
                    )
                ot_sb = outsp.tile([128, GW], F32, tag="otsb")
                nc.vector.tensor_copy(ot_sb[:, :], ot_ps[:, :])
                nc.sync.dma_start(outT_d[:, i0 * 128:i0 * 128 + GW],
                                  ot_sb[:, :])

            nc.sync.dma_start(den_d[:, :], den_sb[:, :])

    nc.compile()
    return nc


_CACHED = {}


def _get_graph():
    if "nc" not in _CACHED:
        _CACHED["nc"] = build_graph()
    return _CACHED["nc"]


def _prepare_inputs(q, k, v, perm):
    q = np.asarray(q, dtype=np.float32)
    k = np.asarray(k, dtype=np.float32)
    v = np.asarray(v, dtype=np.float32)
    perm = np.asarray(perm, dtype=np.int64)

    def rearr(x):  # [1,8,SEQ,D] -> video permuted first, text appended
        return np.concatenate([x[0, :, TEXT:, :][:, perm, :], x[0, :, :TEXT, :]],
                              axis=1)

    qr, kr, vr = rearr(q), rearr(k), rearr(v)      # [8, SEQ, D]
    i31 = np.eye(31, dtype=np.float32)
    # basis: bas[r, j, m] = 1 if m == j and key row r of block j is valid
    bas = np.zeros((128, NB, NB), np.float16)
    for j in range(NB):
        kv = NVLAST if j == 30 else 128
        bas[:kv, j, j] = 1.0
    bas = np.ascontiguousarray(bas.reshape(128, NB * NB))
    in_maps = []
    for c in range(NCORES):
        qp = np.zeros((SP, D), np.float16)
        qp[:SEQ] = qr[c]
        kp = np.zeros((SP, D), np.float16)
        kp[:SEQ] = kr[c]
        vp = np.zeros((SP, D), np.float16)
        vp[:SEQ] = vr[c]
        in_maps.append({
            "qT": np.ascontiguousarray(qp.T),
            "kT": np.ascontiguousarray(kp.T),
            "vv": np.ascontiguousarray(
                vp.reshape(NB, 128, D).transpose(1, 0, 2).reshape(128, NB * 128)),
            "bas": bas,
            "i31": i31,
        })
    return in_maps, perm


def run(inputs, trace=False, trace_kwargs=None):
    nc = _get_graph()
    in_maps, perm = _prepare_inputs(inputs["q"], inputs["k"], inputs["v"],
                                    inputs["perm"])
    res = run_bass_kernel_spmd(
        nc, in_maps, core_ids=list(range(NCORES)), trace=trace,
        **(trace_kwargs or {}),
    )
    outs = np.empty((NCORES, SEQ, D), np.float32)
    for c in range(NCORES):
        oT = res.results[c]["outT"][:, :SEQ]          # [D, SEQ]
        den = res.results[c]["den"][0, :SEQ]          # [SEQ]
        outs[c] = (oT / den[None, :]).T
    g2o = np.argsort(perm)
    txt = outs[:, VID:SEQ, :]
    vid = outs[:, :VID, :][:, g2o, :]
    full = np.concatenate([txt, vid], axis=1)[None]   # [1, 8, SEQ, D]
    return np.ascontiguousarray(full.astype(np.float32)), res


def kernel(q, k, v, perm):
    out, _ = run({"q": q, "k": k, "v": v, "perm": perm})
    return out
